# revision 1
# baseline (speedup 1.0000x reference)
"""Builder for the AttnBlock Trainium2 kernel.

Layout strategy (per core: NB batches of NT tokens, C=512 channels):
  - LN1 computed token-major (bn_stats over free axis), h cast to bf16
  - h transposed to feature-major hT (DMA transpose by default; matmul
    contracts over the partition axis so both operands need C on partitions)
  - QKV projection split: q computed feature-major (qT = w_q^T @ hT),
    k/v computed token-major (kv = hT^T @ w_kv)
  - q softmax over d: exp on ACT during psum->sbuf copy; per-(token,head)
    sums via a packed ones-matmul; normalization applied by replicating
    1/S_q across partitions with a K=2 matmul and one DVE multiply
  - k softmax over n: exp only; the denominator S_k[d] = sum_n e_k[n,d]
    falls out of the context matmul via an appended ones-column on v
  - context[h] = e_k[h]^T @ [v[h] | 1] accumulated per 512-token chunk in
    PSUM (two heads packed in array column halves), folded into an SBUF
    accumulator; rows scaled by 1/(S_k * NT * 8) at bf16 cast
  - attn^T = context^T @ qnorm per head, two heads packed in diagonal
    array quadrants (partitions 0-63 / 64-127)
  - y = attn @ w_out token-major (+ b_out if nonzero), LN2 straight from
    PSUM, (* ln2_scale if non-unit), + x, DMA out
"""

import functools
from contextlib import ExitStack

import ml_dtypes
import numpy as np

import concourse.bass as bass
import concourse.bacc as bacc
import concourse.mybir as mybir
import concourse.tile as tile
import concourse.hw_specs as _hw_specs

# --- activation-table steering -------------------------------------------
# The kernel's only transcendentals are Exp and Ln (rsqrt == exp(-0.5*ln)).
# Both live together in the 'natural_log_exp_and_others' set, but the
# table-load placement pass pairs Exp with 'exp_and_others' and Ln with
# 'natural_log', thrashing the ACT table RAM (~2.7us per switch).  Strip
# Exp/Ln from every other set (membership only — dict order, and hence
# act_func_set_id numbering, is preserved) so the combined set is the only
# candidate and exactly one load is emitted.
_orig_get_activation_tables = _hw_specs.get_activation_tables


@functools.cache
def _steered_activation_tables(module_arch):
    tabs = {k: set(v) for k, v in _orig_get_activation_tables(module_arch).items()}
    combo = "natural_log_exp_and_others"
    if combo in tabs:
        af = mybir.ActivationFunctionType
        for name, fns in tabs.items():
            if name != combo:
                fns.discard(af.Exp)
                fns.discard(af.Ln)
    return tabs


_hw_specs.get_activation_tables = _steered_activation_tables
bacc.get_activation_tables = _steered_activation_tables

P = 128
HEADS = 8
DHEAD = 64
C = 512
DIM = 512
F_QKV = 3 * DIM
EPS = 1e-5

FP32 = mybir.dt.float32
BF16 = mybir.dt.bfloat16
AF = mybir.ActivationFunctionType
ALU = mybir.AluOpType


def build_nc(n_b=2, n_tok=4096, use_bout=False, use_s2=False,
             transpose_mode="pe", pack_quadrants=True, rsqrt_mode="lnexp",
             vext_engine="vector", attn_engine="vec", mm_bufs=4, repeat=1, stage="full", ctx_bufs=2, sq_bufs=1, rep_bufs=1, fp8=True,
             ln1_batch=False, ln2_batch=False, p2_pipe=1, ht_engine="vec"):
    """Build + compile the Bacc graph for one core handling [n_b, n_tok, C]."""
    nc = bacc.Bacc(
        "TRN2", target_bir_lowering=False, debug=False, enable_asserts=False
    )
    # x arrives host-precast to bf16 (bn_stats/h_tm run in the DVE 2x mode,
    # x-load DMA is half-width); the residual "+ x" is applied on the host,
    # so the device only returns the LN2 output z in bf16
    xbf_d = nc.dram_tensor("x_bf", [n_b, n_tok, C], BF16,
                           kind="ExternalInput").ap()
    wqkv_d = nc.dram_tensor("w_qkv", [C, F_QKV], FP32, kind="ExternalInput").ap()
    wout_d = nc.dram_tensor("w_out", [DIM, C], FP32, kind="ExternalInput").ap()
    bout_d = nc.dram_tensor("b_out", [C], FP32, kind="ExternalInput").ap()
    s2_d = nc.dram_tensor("ln2_scale", [C], FP32, kind="ExternalInput").ap()
    out_d = nc.dram_tensor("out", [n_b, n_tok, C], BF16, kind="ExternalOutput").ap()

    with tile.TileContext(nc) as tc:
        _body(tc, xbf_d, wqkv_d, wout_d, bout_d, s2_d, out_d, n_b, n_tok,
              use_bout, use_s2, transpose_mode, pack_quadrants, rsqrt_mode,
              vext_engine, attn_engine, mm_bufs, repeat, stage, ctx_bufs,
              sq_bufs, rep_bufs, fp8, ln1_batch, ln2_batch, p2_pipe, ht_engine)
    nc.compile()
    return nc


def _body(tc, xbf_d, wqkv_d, wout_d, bout_d, s2_d, out_d, n_b, n_tok,
          use_bout, use_s2, transpose_mode, pack_quadrants, rsqrt_mode,
          vext_engine, attn_engine, mm_bufs, repeat=1, stage="full",
          ctx_bufs=2, sq_bufs=1, rep_bufs=1, fp8=False,
          ln1_batch=True, ln2_batch=True, p2_pipe=1, ht_engine="vec"):

    def rsqrt(nc, out, var_ap, eps_t, scale=1.0, power=-0.5):
        # 1/sqrt(scale*var+eps) (power=-0.5) or 1/(scale*var) (power=-1)
        if rsqrt_mode == "lnexp":
            nc.scalar.activation(out, var_ap, AF.Ln, bias=eps_t, scale=scale)
            nc.scalar.activation(out, out, AF.Exp, scale=power)
        else:
            if power == -1.0:
                nc.scalar.mul(out, var_ap, scale)
                nc.vector.reciprocal(out, out)
            else:
                nc.scalar.activation(out, var_ap, AF.Sqrt, bias=eps_t,
                                     scale=scale)
                nc.vector.reciprocal(out, out)
    nc = tc.nc
    NCH = n_tok // 512          # 512-token chunks per batch
    CTX_SCALE = float(n_tok) * 8.0  # v/n and q/sqrt(dhead) folded together
    FP8 = mybir.dt.float8e4
    MMDT = FP8 if fp8 else BF16
    DR = mybir.MatmulPerfMode.DoubleRow if fp8 else None
    W_SC = 32.0 if fp8 else 1.0        # weight pre-scale into fp8 range
    CTX_UP = 1.0
    QN_UP = 1.0
    Y_DESC = 1.0 / (CTX_UP * QN_UP * W_SC)  # undo boosts after y matmul

    with ExitStack() as ctx:
        consts = ctx.enter_context(tc.tile_pool(name="consts", bufs=1))
        work = ctx.enter_context(tc.tile_pool(name="work", bufs=3))
        big = ctx.enter_context(tc.tile_pool(name="big", bufs=2))
        psum = ctx.enter_context(tc.tile_pool(name="psum", bufs=1, space="PSUM"))

        # ---- constants / weights ----
        if transpose_mode == "pe":
            # inline identity via the sync queue: keeps the first PE
            # transposes off the gpsimd queue that carries 4MB of weights
            id_np = np.eye(P, dtype=ml_dtypes.bfloat16)
            ident = consts.tile([P, P], BF16)
            nc.sync.dma_start(ident[:], nc.inline_tensor(id_np, "ident").ap())

        # w_qkv fp32 [c, f] -> bf16 SBUF, c on partitions in 4 chunks
        wq_f = consts.tile([P, 4, DIM], FP32)
        wkv_f = consts.tile([P, 4, 2 * DIM], FP32)
        wo_f = consts.tile([P, 4, C], FP32)
        # weight loads ride the gpsimd (SWDGE) queue so the first x tiles
        # don't wait behind 4MB of weights on the sync queue
        wq_r = wqkv_d.rearrange("(k p) f -> p k f", p=P)
        nc.gpsimd.dma_start(wq_f[:], wq_r[:, :, 0:DIM])
        nc.gpsimd.dma_start(wkv_f[:], wq_r[:, :, DIM:3 * DIM])
        nc.gpsimd.dma_start(wo_f[:], wout_d.rearrange("(k p) f -> p k f", p=P))
        wq = consts.tile([P, 4, DIM], MMDT)
        wkv = consts.tile([P, 4, 2 * DIM], MMDT)
        wo = consts.tile([P, 4, C], BF16)
        nc.vector.tensor_copy(wo[:], wo_f[:])
        if fp8:
            nc.vector.tensor_scalar_mul(wq[:], wq_f[:], W_SC)
            nc.vector.tensor_scalar_mul(wkv[:], wkv_f[:], W_SC)
        else:
            nc.vector.tensor_copy(wq[:], wq_f[:])
            nc.vector.tensor_copy(wkv[:], wkv_f[:])

        if use_bout:
            bout_bc = consts.tile([P, C], FP32)
            nc.sync.dma_start(bout_bc[:], bout_d[None, :].partition_broadcast(P))
        if use_s2:
            s2_bc = consts.tile([P, C], FP32)
            nc.sync.dma_start(s2_bc[:], s2_d[None, :].partition_broadcast(P))
        eps_t = consts.tile([P, 1], FP32)
        nc.vector.memset(eps_t[:], EPS)

        # S_q sums for all 4 m-tiles share one PSUM bank, written at
        # partition stripes {0,32,64,96}+{0,1} via tile_position col-tiling.
        # m=0 uses a 98-col lhsT: cols 0/1 are the head-pair indicators,
        # cols at the other stripes are ZERO (so later m's accumulate onto
        # zero), remaining cols ONE (so every row 0..97 is written and the
        # single [98,512] reciprocal sees no uninitialized psum).
        sq0_np = np.ones((P, 98), ml_dtypes.bfloat16)
        sq0_np[:, 0] = 0.0
        sq0_np[:, 1] = 0.0
        sq0_np[0:64, 0] = 1.0 / QN_UP
        sq0_np[64:128, 1] = 1.0 / QN_UP
        for _m in (1, 2, 3):
            sq0_np[:, 32 * _m] = 0.0
            sq0_np[:, 32 * _m + 1] = 0.0
        sq0_ones = consts.tile([P, 98], BF16)
        nc.sync.dma_start(sq0_ones[:], nc.inline_tensor(sq0_np, "sq0_ones").ap())
        hp_np = np.zeros((P, 2), ml_dtypes.bfloat16)
        hp_np[0:64, 0] = 1.0 / QN_UP
        hp_np[64:128, 1] = 1.0 / QN_UP
        hp_ones = consts.tile([P, 2], BF16)
        nc.sync.dma_start(hp_ones[:], nc.inline_tensor(hp_np, "hp_ones").ap())
        # per-m selector for replicating rq8 stripes -> [128, t]: lhsT [98, 128]
        sel_tiles = []
        sel_np = np.zeros((4, 98, P), ml_dtypes.bfloat16)
        for _m in range(4):
            sel_np[_m, 32 * _m, 0:64] = 1
            sel_np[_m, 32 * _m + 1, 64:128] = 1
        for _m in range(4):
            st = consts.tile([98, P], BF16, tag=f"sel{_m}")
            nc.sync.dma_start(st[:], nc.inline_tensor(
                np.ascontiguousarray(sel_np[_m]), f"sel{_m}").ap())
            sel_tiles.append(st)

        # persistent vext tiles: the trailing ones-column (for S_k via the
        # context matmul) is written once and survives v overwrites
        N_VEXT = 9
        vext_tiles = []
        for i in range(N_VEXT):
            vt = work.tile([P, HEADS, DHEAD + 1], BF16, tag=f"vext{i}", bufs=1)
            nc.vector.memset(vt[:, :, DHEAD:DHEAD + 1], 1.0)
            vext_tiles.append(vt)
        vext_idx = 0

        rep_cm = tc.For_i(
            0, repeat, 1,
            hint_engines=(mybir.EngineType.PE, mybir.EngineType.DVE,
                          mybir.EngineType.Activation),
        ) if repeat > 1 else None
        if rep_cm is not None:
            rep_cm.__enter__()
        # per-batch persistent tiles, both batches in flight (chunk-interleaved)
        expq_b = []
        ctx_ps_b = []
        ctx_bf_b = []
        for b in range(n_b):
            expq = big.tile([P, 4, NCH, 512], BF16, tag="expq")
            # context accumulates directly in one PSUM bank across all of
            # pass 1; 128-float stride per head pair = exactly one 2KB bank
            # row, so the has_written zero-region granularity lines up
            ctx_ps = psum.tile([P, 4, 128], FP32, tag=f"ctxp{b}", bufs=1)
            expq_b.append(expq)
            ctx_ps_b.append(ctx_ps)

        # ---------------- pass 1: 1-chunk software pipeline ------------
        # The LN1 stage of chunk c+1 (x DMA, stats, rsqrt, h_tm, transposes)
        # is emitted BEFORE the compute stage of chunk c, so per-engine FIFO
        # order never lets a compute op that waits on a cross-engine result
        # head-block the next chunk's ready LN1 work.
        def ln1_stage(tcn, b):
            hT = big.tile([P, 4, 512], MMDT, tag="hT", bufs=3)
            xts = []
            mv4 = work.tile([P, 4, 2], FP32, tag="bn_mv", bufs=3)
            rstd4 = work.tile([P, 4], FP32, tag="rstd", bufs=3)
            for ti in range(4):
                t0 = tcn * 512 + ti * 128
                xt = work.tile([P, C], BF16, tag="x_in", bufs=10)
                nc.sync.dma_start(xt[:], xbf_d[b, t0:t0 + 128, :])
                stats = work.tile([P, 6], FP32, tag="bn_st", bufs=6)
                nc.vector.bn_stats(stats[:], xt[:])
                nc.vector.bn_aggr(mv4[:, ti, :], stats[:])
                if not ln1_batch:
                    rsqrt(nc, rstd4[:, ti:ti + 1], mv4[:, ti, 1:2], eps_t[:])
                xts.append(xt)
            if ln1_batch:
                rsqrt(nc, rstd4[:], mv4[:, :, 1], eps_t[:])
            # all h_tm ops first so a pending transpose drain never
            # head-blocks a ready h_tm in the DVE FIFO
            h_tms = []
            for ti in range(4):
                h_tm = work.tile([P, C], BF16, tag="h_tm", bufs=6)
                nc.vector.tensor_scalar(
                    out=h_tm[:], in0=xts[ti], scalar1=mv4[:, ti, 0:1],
                    scalar2=rstd4[:, ti:ti + 1], op0=ALU.subtract,
                    op1=ALU.mult)
                h_tms.append(h_tm)
            for ti in range(4):
                # 4 transposes into one psum tile, drained by one copy
                ps_tp = psum.tile([P, 4, P], BF16, tag="mm", bufs=mm_bufs)
                for ck in range(4):
                    nc.tensor.transpose(ps_tp[:, ck, :],
                                        h_tms[ti][:, ck * P:(ck + 1) * P],
                                        ident[:])
                if ht_engine == "act":
                    nc.scalar.copy(
                        hT[:, :, ti * 128:(ti + 1) * 128], ps_tp[:])
                else:
                    nc.vector.tensor_copy(
                        hT[:, :, ti * 128:(ti + 1) * 128], ps_tp[:])
            return hT

        def compute_stage(tcn, b, hT):
            nonlocal vext_idx
            expq = expq_b[b]
            ctx_ps = ctx_ps_b[b]
            ek_t = []
            vext_t = []
            # q part: feature-major, 4 m-tiles of 128 dims (= head pairs)
            eqs = []
            ps_sq8 = psum.tile([P, 512], FP32, tag="sqrep", bufs=2)
            for m in range(4):
                ps_q = psum.tile([P, 512], FP32, tag="mm", bufs=mm_bufs)
                if fp8:
                    for k2 in (0, 2):
                        nc.tensor.matmul(
                            ps_q[:], wq[:, k2:k2 + 2, m * 128:(m + 1) * 128],
                            hT[:, k2:k2 + 2, :], start=(k2 == 0),
                            stop=(k2 == 2), perf_mode=DR)
                else:
                    for k in range(4):
                        nc.tensor.matmul(
                            ps_q[:], wq[:, k, m * 128:(m + 1) * 128],
                            hT[:, k, :], start=(k == 0), stop=(k == 3))
                eq = expq[:, m, tcn, :]
                nc.scalar.activation(eq, ps_q[:], AF.Exp, scale=1.0 / W_SC)
                eqs.append(eq)
            # per-(token, head) sums over d, emitted after ALL q matmuls
            # so a sum waiting on exp(m) never head-blocks ready q work
            # in the PE FIFO; all 4 m-tiles land in ONE bank at partition
            # stripes {32m, 32m+1}
            for m in range(4):
                if m == 0:
                    nc.tensor.matmul(ps_sq8[0:98, :], sq0_ones[:], eqs[0],
                                     start=True, stop=False,
                                     skip_group_check=True)
                else:
                    nc.tensor.matmul(ps_sq8[32 * m:32 * m + 2, :],
                                     hp_ones[:], eqs[m],
                                     start=False, stop=(m == 3),
                                     tile_position=(0, 32 * m),
                                     skip_group_check=True)
            # one reciprocal covers all 4 m-tiles' S_q rows
            rq8 = work.tile([98, 512], BF16, tag="rq8", bufs=2)
            with nc.allow_low_precision(reason="1/S_q in bf16 is fine"):
                nc.vector.reciprocal(rq8[:], ps_sq8[0:98, :])
            for m in range(4):
                ps_rep = psum.tile([P, 512], FP32, tag="sqrep", bufs=2)
                nc.tensor.matmul(ps_rep[:], sel_tiles[m][:], rq8[:],
                                 start=True, stop=True)
                nc.vector.tensor_tensor(eqs[m], eqs[m], ps_rep[:], ALU.mult)

            # k/v part: token-major [128t, 512f]
            for ti in range(4):
                ek = work.tile([P, 512], BF16, tag="ek", bufs=8)
                ps_k = psum.tile([P, 512], FP32, tag="mm", bufs=mm_bufs)
                if fp8:
                    for k2 in (0, 2):
                        nc.tensor.matmul(
                            ps_k[:], hT[:, k2:k2 + 2, ti * 128:(ti + 1) * 128],
                            wkv[:, k2:k2 + 2, 0:512], start=(k2 == 0),
                            stop=(k2 == 2), perf_mode=DR)
                else:
                    for k in range(4):
                        nc.tensor.matmul(
                            ps_k[:], hT[:, k, ti * 128:(ti + 1) * 128],
                            wkv[:, k, 0:512], start=(k == 0), stop=(k == 3))
                nc.scalar.activation(ek[:], ps_k[:], AF.Exp,
                                     scale=1.0 / W_SC)
                ek_t.append(ek)

                vext = vext_tiles[vext_idx % N_VEXT]
                vext_idx += 1
                ps_v = psum.tile([P, 512], FP32, tag="mm", bufs=mm_bufs)
                if fp8:
                    for k2 in (0, 2):
                        nc.tensor.matmul(
                            ps_v[:], hT[:, k2:k2 + 2, ti * 128:(ti + 1) * 128],
                            wkv[:, k2:k2 + 2, 512:1024], start=(k2 == 0),
                            stop=(k2 == 2), perf_mode=DR)
                else:
                    for k in range(4):
                        nc.tensor.matmul(
                            ps_v[:], hT[:, k, ti * 128:(ti + 1) * 128],
                            wkv[:, k, 512:1024], start=(k == 0), stop=(k == 3))
                if vext_engine == "act":
                    nc.scalar.mul(
                        vext[:, :, 0:DHEAD],
                        ps_v.rearrange("p (h e) -> p h e", h=HEADS),
                        1.0 / W_SC)
                else:
                    nc.vector.tensor_scalar_mul(
                        vext[:, :, 0:DHEAD],
                        ps_v.rearrange("p (h e) -> p h e", h=HEADS),
                        1.0 / W_SC)
                vext_t.append(vext)

            if stage == "qkv":
                return
            # context accumulates in ctx_ps (one PSUM bank per batch)
            # across ALL chunks of pass 1: the first matmul of each
            # partition-half group uses start=True, the very last stop=True.
            first_cx = (tcn == 0)
            last_cx = (tcn == NCH - 1)
            for hp in range(4):
                for ti in range(4):
                    ek = ek_t[ti]
                    he, ho = 2 * hp, 2 * hp + 1
                    nc.tensor.matmul(
                        ctx_ps[0:64, hp, 0:DHEAD + 1], ek[:, he * 64:he * 64 + 64],
                        vext_t[ti][:, he, :],
                        start=(first_cx and hp == 0 and ti == 0),
                        stop=False,
                        tile_position=(0, 0) if pack_quadrants else None,
                        skip_group_check=True)
                    nc.tensor.matmul(
                        ctx_ps[64:128, hp, 0:DHEAD + 1], ek[:, ho * 64:ho * 64 + 64],
                        vext_t[ti][:, ho, :],
                        start=(first_cx and hp == 0 and ti == 0),
                        stop=(last_cx and hp == 3 and ti == 3),
                        tile_position=(0, 64) if pack_quadrants else None,
                        skip_group_check=True)

        p1_order = [divmod(tcn_b, n_b) for tcn_b in range(NCH * n_b)]
        hts = {p1_order[0]: ln1_stage(*p1_order[0])}
        for i, cb in enumerate(p1_order):
            if i + 1 < len(p1_order):
                nxt = p1_order[i + 1]
                hts[nxt] = ln1_stage(*nxt)
            if stage != "ln1":
                compute_stage(cb[0], cb[1], hts.pop(cb))

        if stage in ("ln1", "qkv", "p1"):
            if rep_cm is not None:
                rep_cm.__exit__(None, None, None)
            return
        # ---------------- context finalize ----------------
        for b in range(n_b):
            ctx_ps = ctx_ps_b[b]
            ctx_bf = big.tile([P, 4, DHEAD], BF16, tag="ctx_bf")
            ctx_bf_b.append(ctx_bf)
            # one batched 1/(CTX_SCALE * S_k) for all 4 head pairs
            s_col4 = work.tile([P, 4], FP32, tag="sk", bufs=2)
            rsqrt(nc, s_col4[:], ctx_ps[:, :, DHEAD], eps_t[:],
                  scale=CTX_SCALE / CTX_UP, power=-1.0)
            for hp in range(4):
                nc.vector.tensor_scalar_mul(
                    ctx_bf[:, hp, :], ctx_ps[:, hp, 0:DHEAD],
                    s_col4[:, hp:hp + 1])

        # ---------------- pass 2: 1-chunk software pipeline ------------
        # attention stage of chunk c+1 emitted before the y/LN2 stage of
        # chunk c, for the same FIFO head-blocking reason as pass 1
        def at_stage(tcn, b):
            expq = expq_b[b]
            ctx_bf = ctx_bf_b[b]
            at8 = work.tile([P, 4, 512], BF16, tag="attn", bufs=3)
            for hp in range(4):
                ps_at = psum.tile([P, 512], FP32, tag="mm", bufs=mm_bufs)
                nc.tensor.matmul(
                    ps_at[0:64, :], ctx_bf[0:64, hp, :],
                    expq[0:64, hp, tcn, :], start=True, stop=True,
                    tile_position=(0, 0) if pack_quadrants else None,
                    skip_group_check=True)
                nc.tensor.matmul(
                    ps_at[64:128, :], ctx_bf[64:128, hp, :],
                    expq[64:128, hp, tcn, :], start=True, stop=True,
                    tile_position=(64, 64), skip_group_check=True)
                if attn_engine == "act":
                    nc.scalar.copy(at8[:, hp, :], ps_at[:])
                else:
                    nc.vector.tensor_copy(at8[:, hp, :], ps_at[:])
            return at8

        def y_stage(tcn, b, at8):
            # y matmul kept in PSUM; LN2 stats read PSUM directly; rsqrt
            # either batched over the 4 token tiles or per-tile.  The four
            # z tiles live in one [P, 4, C] tile flushed by a single DMA
            # (64 -> 16 SWDGE triggers for the whole output).
            ps_ys = []
            mv2 = work.tile([P, 4, 2], FP32, tag="bn_mv2", bufs=2)
            r2_4 = work.tile([P, 4], FP32, tag="r2", bufs=2)
            nmr2_4 = work.tile([P, 4], FP32, tag="nmr2", bufs=2)
            z4 = work.tile([P, 4, C], BF16, tag="z", bufs=2)
            for ts in range(4):
                ps_y = psum.tile([P, 512], FP32, tag="mm", bufs=mm_bufs)
                for hp in range(4):
                    nc.tensor.matmul(
                        ps_y[:], at8[:, hp, ts * 128:(ts + 1) * 128],
                        wo[:, hp, :], start=(hp == 0), stop=(hp == 3))
                if use_bout:
                    y_src = work.tile([P, C], FP32, tag="y_sb", bufs=4)
                    nc.vector.tensor_tensor(y_src[:], ps_y[:], bout_bc[:],
                                            ALU.add)
                    src = y_src
                else:
                    src = ps_y
                stats2 = work.tile([P, 6], FP32, tag="bn_st2", bufs=4)
                nc.vector.bn_stats(stats2[:], src[:])
                nc.vector.bn_aggr(mv2[:, ts, :], stats2[:])
                if not ln2_batch:
                    rsqrt(nc, r2_4[:, ts:ts + 1], mv2[:, ts, 1:2], eps_t[:])
                    nc.vector.tensor_scalar(
                        out=nmr2_4[:, ts:ts + 1], in0=mv2[:, ts, 0:1],
                        scalar1=r2_4[:, ts:ts + 1], scalar2=-1.0,
                        op0=ALU.mult, op1=ALU.mult)
                    nc.scalar.activation(z4[:, ts, :], src[:], AF.Identity,
                                         bias=nmr2_4[:, ts:ts + 1],
                                         scale=r2_4[:, ts:ts + 1])
                    if use_s2:
                        nc.vector.tensor_tensor(z4[:, ts, :], z4[:, ts, :],
                                                s2_bc[:], ALU.mult)
                ps_ys.append(src)
            if not ln2_batch:
                out_rr = out_d[b, tcn * 512:(tcn + 1) * 512, :].rearrange(
                    "(t p) c -> p t c", p=P)
                nc.gpsimd.dma_start(out_rr, z4[:])
            if ln2_batch:
                rsqrt(nc, r2_4[:], mv2[:, :, 1], eps_t[:])
                nc.vector.tensor_tensor(nmr2_4[:], mv2[:, :, 0], r2_4[:],
                                        ALU.mult)
                nc.vector.tensor_scalar_mul(nmr2_4[:], nmr2_4[:], -1.0)
                for ts in range(4):
                    t0 = tcn * 512 + ts * 128
                    z = work.tile([P, C], BF16, tag="z", bufs=4)
                    nc.scalar.activation(z[:], ps_ys[ts][:], AF.Identity,
                                         bias=nmr2_4[:, ts:ts + 1],
                                         scale=r2_4[:, ts:ts + 1])
                    if use_s2:
                        nc.vector.tensor_tensor(z[:], z[:], s2_bc[:],
                                                ALU.mult)
                    nc.gpsimd.dma_start(out_d[b, t0:t0 + 128, :], z[:])

        p2_order = [divmod(tcn_b, n_b) for tcn_b in range(NCH * n_b)]
        if p2_pipe:
            at8s = {p2_order[0]: at_stage(*p2_order[0])}
            for i, cb in enumerate(p2_order):
                if i + 1 < len(p2_order):
                    nxt = p2_order[i + 1]
                    at8s[nxt] = at_stage(*nxt)
                y_stage(cb[0], cb[1], at8s.pop(cb))
        else:
            for cb in p2_order:
                y_stage(cb[0], cb[1], at_stage(*cb))

        if rep_cm is not None:
            rep_cm.__exit__(None, None, None)



# ---------------------------------------------------------------------------
# kernel(): full-input entry point. Shards batch over 8 NeuronCores,
# folds ln1_scale into w_qkv on the host, runs the SPMD NEFF, regathers.
# ---------------------------------------------------------------------------

N_CORES = 8
B_FULL = 16
H_IMG = 64
W_IMG = 64
NB_PER_CORE = B_FULL // N_CORES
N_TOK = H_IMG * W_IMG

_nc_cache = {}


def _get_nc(use_bout, use_s2):
    key = (use_bout, use_s2)
    if key not in _nc_cache:
        _nc_cache[key] = build_nc(n_b=NB_PER_CORE, n_tok=N_TOK,
                                  use_bout=use_bout, use_s2=use_s2)
    return _nc_cache[key]


def kernel(x, ln1_scale, w_qkv, w_out, b_out, ln2_scale):
    from concourse.bass_utils import run_bass_kernel_spmd

    x = np.ascontiguousarray(np.asarray(x, dtype=np.float32))
    ln1_scale = np.asarray(ln1_scale, dtype=np.float32)
    w_qkv = np.asarray(w_qkv, dtype=np.float32)
    w_out = np.ascontiguousarray(np.asarray(w_out, dtype=np.float32))
    b_out = np.ascontiguousarray(np.asarray(b_out, dtype=np.float32))
    ln2_scale = np.ascontiguousarray(np.asarray(ln2_scale, dtype=np.float32))

    # fold ln1_scale into the qkv weight (h*s1 @ w == h @ (s1[:,None]*w))
    w_eff = np.ascontiguousarray(ln1_scale[:, None] * w_qkv)

    use_bout = bool(np.any(b_out))
    use_s2 = not bool(np.all(ln2_scale == 1.0))
    nc = _get_nc(use_bout, use_s2)

    xr = x.reshape(B_FULL, N_TOK, C)
    xr_bf = xr.astype(ml_dtypes.bfloat16)
    in_maps = []
    for i in range(N_CORES):
        sl = slice(i * NB_PER_CORE, (i + 1) * NB_PER_CORE)
        in_maps.append({
            "x_bf": np.ascontiguousarray(xr_bf[sl]),
            "w_qkv": w_eff,
            "w_out": w_out,
            "b_out": b_out,
            "ln2_scale": ln2_scale,
        })

    res = run_bass_kernel_spmd(nc, in_maps, core_ids=list(range(N_CORES)))
    z = np.concatenate([r["out"] for r in res.results], axis=0)
    # residual add on the host: out = LN2(y) + x
    out = z.astype(np.float32).reshape(B_FULL, H_IMG, W_IMG, C) + x
    return out.astype(np.float32)

